# revision 18
# baseline (speedup 1.0000x reference)
"""Blended-MoE 3-layer MLP (moe_routing) on 8 trn2 NeuronCores.

Math: per layer  z[b,o] = sum_e blend[e,b] * (w[e] @ h[b] + bias[e])[o],
ELU between layers.  Rewritten as a single contraction per layer:

    z[b,o] = sum_{(e,i)} (blend[e,b] * hT[i,b]) * wT[(e,i), o]
           + sum_e blend[e,b] * bias[e,o]          (bias via one K=8 matmul)

Data-parallel across 8 cores (128 batch rows each); expert weights are
replicated, host-side pre-transposed into SBUF-image layout.

The kernel is DMA-bound (~11.5 MB of weights per core at ~420 GB/s), so the
schedule keeps the weight stream the critical path:
  - all DMAs ride the sync HW-DGE queue in strict consumption order
    (small inputs, blend broadcast, then the weight groups)
  - layers 1/2 use an it-half-major K order so each half of the previous
    layer's output can be ELU'd / transposed / expanded while the other
    half's matmuls still run (short tensor-idle gap at layer boundaries)
  - the Tile kernel tail (drain with per-semaphore waits + 2 barriers +
    clears, ~8 us) is replaced by barrier + range drain + range clear
"""

import numpy as np
import ml_dtypes

import concourse.bass as bass
import concourse.mybir as mybir
import concourse.tile as tile
from concourse.bass_utils import run_bass_kernel_spmd
from concourse.masks import make_identity

import bass_rust

# ---- config ----------------------------------------------------------------
N_CORES = 8
B, E = 1024, 8
DIN, D1, D2, D3 = 480, 512, 512, 311
N_L = (D1, D2, D3)
NKT = 32          # k-tiles per layer (8 experts * 4); bias handled separately
GSIZE = 8         # k-tiles per weight-group DMA

PROFILE = {"trace": False, "tmpdir": None}
LAST_RESULT = [None]

_NC_CACHE = {}
_SPLIT_N = [0]


def _ktile_order(layer):
    """Order in which k-tiles (e, it) are consumed by the matmul loop.

    Layer 0's activations (xT) arrive whole from the host, so expert-major
    order matches the natural expansion.  Layers 1/2 consume it-half-major
    so matmuls can start when only the first half of hT exists."""
    if layer == 0:
        return [(e, it) for e in range(E) for it in range(4)]
    return [
        (e, 2 * half + sub)
        for half in range(2)
        for e in range(E)
        for sub in range(2)
    ]


def _split_multi_waits(nc, max_waits=1):
    """This container's walrus only supports one sync-wait command per
    instruction; spill extras onto same-engine NOPs inserted just before."""
    for f in nc.m.functions:
        for bb in f.blocks:
            insts = bb.instructions
            i = 0
            while i < len(insts):
                inst = insts[i]
                si = inst.sync_info
                if si is not None and len(si.on_wait) > max_waits:
                    waits = list(si.on_wait)
                    extra, keep = waits[:-max_waits], waits[-max_waits:]
                    for w in extra:
                        _SPLIT_N[0] += 1
                        nop = mybir.InstNoOp(
                            name=f"wsplit-{_SPLIT_N[0]}", ins=[], outs=[]
                        )
                        nop.engine = inst.engine
                        nop.sync_info = bass_rust.SyncInfo(
                            on_wait=[w], on_update=[]
                        )
                        insts.insert(i, nop)
                        i += 1
                    inst.sync_info = bass_rust.SyncInfo(
                        on_wait=keep, on_update=list(si.on_update)
                    )
                i += 1


class _FastTailTC(tile.TileContext):
    """Tile's kernel tail is drain-with-per-sem-waits + 2 all-engine
    barriers + per-sem clears; the per-sem waits explode into ~70 NOPs per
    engine under the single-wait walrus (~8 us).  All DMAs except the
    output writeback have already been observed by their consumers, so a
    barrier (engines idle, all triggers issued) followed by the range-based
    DMA drain + semaphore clear inside clear_and_free_semaphores is enough."""

    def _drain_and_barrier(self, tick_clock, wait_clock):
        nc = self.nc
        nc.all_engine_barrier()
        popped = nc._tile_sem_poison_stack.pop()
        assert popped is self._sem_poison
        assert self.sems is not None
        nc.clear_and_free_semaphores(list(self.sems.allocated().values()))


def _build_nc():
    f32 = mybir.dt.float32
    dt = mybir.dt.float16
    nc = bass.Bass()

    w_d = [
        nc.dram_tensor(f"w{l}s", [128, NKT * n], dt, kind="ExternalInput")
        for l, n in enumerate(N_L)
    ]
    xt_d = nc.dram_tensor("xt", [128, 512], dt, kind="ExternalInput")
    # [ blT (8x128) | bias0 (8x512) | bias1 (8x512) | bias2 (8x311) ]
    SMALL_COLS = 128 + D1 + D2 + D3
    small_d = nc.dram_tensor("small", [8, SMALL_COLS], dt, kind="ExternalInput")
    # bb[p, e*512 + it*128 + b] = blend[e, b]: DMA-ing this 1 MB broadcast
    # costs ~2.7 us of stream but frees ~4 us of the in-order tensor queue
    # (on-chip K=8 broadcast matmuls measured slower than the DMA)
    bb_d = nc.dram_tensor("bb", [128, E * 512], dt, kind="ExternalInput")
    out_d = nc.dram_tensor("out", [128, D3], f32, kind="ExternalOutput")

    with _FastTailTC(nc) as tc:
        with (
            tc.tile_pool(name="const", bufs=1) as const,
            tc.tile_pool(name="w", bufs=12) as wpool,
            tc.tile_pool(name="acts", bufs=2) as acts,
            tc.tile_pool(name="tmp", bufs=2) as tmp,
            tc.tile_pool(name="zp", bufs=2, space="PSUM") as zp,
            tc.tile_pool(name="tp", bufs=2, space="PSUM") as tp,
        ):
            # everything rides the sync HW-DGE queue: the scalar queue
            # initializes lazily (~3.5 us) and moves 512-byte packets, so
            # it is useless for latency; small inputs go first (~0.6 us)
            small_sb = const.tile([8, SMALL_COLS], dt)
            nc.sync.dma_start(small_sb[:], small_d[:])
            xt_sb = const.tile([128, 512], dt)
            nc.sync.dma_start(xt_sb[:], xt_d[:])

            bl8 = small_sb[:, 0:128]
            bb = const.tile([128, E * 512], dt)
            # first bb half (experts 0-3) ahead of w0g0 so layer-0 expansion
            # overlaps the first weight group's transfer; second half rides
            # between w0g0 and w0g1 (needed only by the third mm group)
            nc.sync.dma_start(bb[:, : E * 256], bb_d[:, : E * 256])
            waug = [None] * 3
            off = 128
            for l, n in enumerate(N_L):
                waug[l] = small_sb[:, off : off + n]
                off += n

            # weight stream: strict consumption order on the sync queue
            wg = [[] for _ in N_L]  # [(tile, start_kt, n_kt)]
            for l, n in enumerate(N_L):
                for g in range(NKT // GSIZE):
                    t = wpool.tile([128, GSIZE * n], dt, tag=f"w{l}g{g}", bufs=1)
                    nc.sync.dma_start(
                        t[:], w_d[l][:, g * GSIZE * n : (g + 1) * GSIZE * n]
                    )
                    wg[l].append((t, g * GSIZE, GSIZE))
                    if l == 0 and g == 0:
                        nc.sync.dma_start(
                            bb[:, E * 256 :], bb_d[:, E * 256 :]
                        )

            ident = const.tile([128, 128], dt)
            make_identity(nc, ident[:])

            def expand(he, src_ap, e, lo, hi):
                # he[:, e*512+lo : e*512+hi] = src_ap * blend[e]
                nc.vector.tensor_tensor(
                    he[:, e * 512 + lo : e * 512 + hi],
                    src_ap,
                    bb[:, e * 512 + lo : e * 512 + hi],
                    mybir.AluOpType.mult,
                )

            he = acts.tile([128, E * 512], dt, tag="he")
            for e in range(E):
                expand(he, xt_sb[:], e, 0, 512)

            for l, n in enumerate(N_L):
                z = zp.tile([128, n], f32, tag="z")
                # bias: z[b, o] = sum_e bl[e,b] * bias[e,o]   (K=8 matmul)
                nc.tensor.matmul(z[:], bl8, waug[l], start=True, stop=False)
                order = _ktile_order(l)
                for j, (e, it) in enumerate(order):
                    g, loc = divmod(j, GSIZE)
                    wt = wg[l][g][0]
                    nc.tensor.matmul(
                        z[:],
                        he[:, e * 512 + it * 128 : e * 512 + (it + 1) * 128],
                        wt[:, loc * n : (loc + 1) * n],
                        start=False,
                        stop=(j == NKT - 1),
                    )

                if l == 2:
                    # output: two halves so copy/trigger/data overlap
                    out_sb = tmp.tile([128, D3], f32, tag="osb")
                    nc.scalar.copy(out_sb[:, 0:160], z[:, 0:160])
                    nc.sync.dma_start(out_d[:, 0:160], out_sb[:, 0:160])
                    nc.vector.tensor_copy(out_sb[:, 160:D3], z[:, 160:D3])
                    nc.sync.dma_start(out_d[:, 160:D3], out_sb[:, 160:D3])
                    break

                # boundary: ELU + transpose + expansion, one 256-col half at
                # a time so layer l+1 matmuls start after the first half.
                he_next = acts.tile([128, E * 512], dt, tag="he")
                for half in range(2):
                    lo, hi = half * 256, (half + 1) * 256
                    m = tmp.tile([128, 256], f32, tag=f"m{half}")
                    nc.vector.tensor_scalar(
                        m[:], z[:, lo:hi], 0.0, None, mybir.AluOpType.min
                    )
                    ex = tmp.tile([128, 256], f32, tag=f"ex{half}")
                    nc.scalar.activation(
                        ex[:], m[:], mybir.ActivationFunctionType.Exp
                    )
                    p = tmp.tile([128, 256], f32, tag=f"p{half}")
                    nc.vector.tensor_scalar(
                        p[:], z[:, lo:hi], 0.0, -1.0,
                        mybir.AluOpType.max, mybir.AluOpType.add,
                    )
                    h = tmp.tile([128, 256], dt, tag=f"h{half}")
                    nc.vector.tensor_tensor(
                        h[:], p[:], ex[:], mybir.AluOpType.add
                    )
                    tps = tp.tile([128, 256], dt, tag=f"tps{half}", bufs=1)
                    for a in range(2):
                        nc.tensor.transpose(
                            tps[:, a * 128 : (a + 1) * 128],
                            h[:, a * 128 : (a + 1) * 128],
                            ident[:],
                        )
                    hT = tmp.tile([128, 256], dt, tag=f"hT{half}")
                    nc.scalar.copy(hT[:], tps[:])
                    for e in range(E):
                        expand(he_next, hT[:], e, lo, hi)
                he = he_next

    _split_multi_waits(nc)
    return nc


# ---- host-side packing -----------------------------------------------------


def _wimg(w, layer, np_dt):
    """(E, dout, din) weights -> (128, 32*dout) SBUF image in the k-tile
    order consumed by the matmul loop of this layer."""
    e_, dout, din = w.shape
    wt = np.zeros((E, 4 * 128, dout), np.float32)
    for e in range(e_):
        wt[e, :din] = w[e].T
    order = _ktile_order(layer)
    img = np.empty((128, NKT * dout), np.float32)
    for j, (e, it) in enumerate(order):
        img[:, j * dout : (j + 1) * dout] = wt[e, it * 128 : (it + 1) * 128]
    return np.ascontiguousarray(img).astype(np_dt)


def kernel(x, weight_blend, w0, b0, w1, b1, w2, b2):
    np_dt = np.float16

    if "nc" not in _NC_CACHE:
        _NC_CACHE["nc"] = _build_nc()
    nc = _NC_CACHE["nc"]

    x = np.asarray(x, np.float32)
    weight_blend = np.asarray(weight_blend, np.float32)
    wimgs = {
        "w0s": _wimg(np.asarray(w0), 0, np_dt),
        "w1s": _wimg(np.asarray(w1), 1, np_dt),
        "w2s": _wimg(np.asarray(w2), 2, np_dt),
    }
    biases = [np.asarray(b, np.float32) for b in (b0, b1, b2)]

    bc = B // N_CORES
    in_maps = []
    for c in range(N_CORES):
        sl = slice(c * bc, (c + 1) * bc)
        xT = np.zeros((4 * 128, bc), np.float32)
        xT[:DIN] = x[sl].T
        xt_img = xT.reshape(4, 128, bc).transpose(1, 0, 2).reshape(128, 4 * bc)
        bl = weight_blend[:, sl]  # (8, 128)
        small_img = np.concatenate([bl] + biases, axis=1)
        bb_img = np.broadcast_to(
            bl[:, None, None, :], (E, 4, 128, bc)
        ).transpose(2, 0, 1, 3).reshape(128, E * 4 * bc)
        in_maps.append(
            {
                **wimgs,
                "xt": np.ascontiguousarray(xt_img).astype(np_dt),
                "small": np.ascontiguousarray(small_img).astype(np_dt),
                "bb": np.ascontiguousarray(bb_img).astype(np_dt),
            }
        )

    res = run_bass_kernel_spmd(
        nc,
        in_maps,
        core_ids=list(range(N_CORES)),
        trace=PROFILE["trace"],
        tmpdir=PROFILE["tmpdir"],
    )
    LAST_RESULT[0] = res
    return np.concatenate(
        [res.results[c]["out"] for c in range(N_CORES)], axis=0
    )


# revision 19
# speedup vs baseline: 1.0643x; 1.0643x over previous
"""Blended-MoE 3-layer MLP (moe_routing) on 8 trn2 NeuronCores.

Math: per layer  z[b,o] = sum_e blend[e,b] * (w[e] @ h[b] + bias[e])[o],
ELU between layers.  Rewritten as a single contraction per layer:

    z[b,o] = sum_{(e,i)} (blend[e,b] * hT[i,b]) * wT[(e,i), o]
           + sum_e blend[e,b] * bias[e,o]          (bias via one K=8 matmul)

Data-parallel across 8 cores (128 batch rows each); expert weights are
replicated, host-side pre-transposed into SBUF-image layout.

The kernel is DMA-bound (~11.5 MB of weights per core at ~420 GB/s), so the
schedule keeps the weight stream the critical path:
  - all DMAs ride the sync HW-DGE queue in strict consumption order
    (small inputs, blend broadcast, then the weight groups)
  - layers 1/2 use an it-half-major K order so each half of the previous
    layer's output can be ELU'd / transposed / expanded while the other
    half's matmuls still run (short tensor-idle gap at layer boundaries)
  - the Tile kernel tail (drain with per-semaphore waits + 2 barriers +
    clears, ~8 us) is replaced by barrier + range drain + range clear
"""

import numpy as np

import concourse.bass as bass
import concourse.mybir as mybir
import concourse.tile as tile
from concourse.bass_utils import run_bass_kernel_spmd
from concourse.masks import make_identity

import bass_rust

# ---- config ----------------------------------------------------------------
N_CORES = 8
B, E = 1024, 8
DIN, D1, D2, D3 = 480, 512, 512, 311
N_L = (D1, D2, D3)
NKT = 32          # k-tiles per layer (8 experts * 4); bias handled separately
GSIZE = 8         # k-tiles per weight-group DMA

PROFILE = {"trace": False, "tmpdir": None}
LAST_RESULT = [None]

_NC_CACHE = {}
_SPLIT_N = [0]


def _ktile_order(layer):
    """Order in which k-tiles (e, it) are consumed by the matmul loop.

    Layer 0's activations (xT) arrive whole from the host, so expert-major
    order matches the natural expansion.  Layers 1/2 consume it-half-major
    so matmuls can start when only the first half of hT exists."""
    if layer == 0:
        return [(e, it) for e in range(E) for it in range(4)]
    return [
        (e, 2 * half + sub)
        for half in range(2)
        for e in range(E)
        for sub in range(2)
    ]


def _split_multi_waits(nc, max_waits=1):
    """This container's walrus only supports one sync-wait command per
    instruction; spill extras onto same-engine NOPs inserted just before."""
    for f in nc.m.functions:
        for bb in f.blocks:
            insts = bb.instructions
            i = 0
            while i < len(insts):
                inst = insts[i]
                si = inst.sync_info
                if si is not None and len(si.on_wait) > max_waits:
                    waits = list(si.on_wait)
                    extra, keep = waits[:-max_waits], waits[-max_waits:]
                    for w in extra:
                        _SPLIT_N[0] += 1
                        nop = mybir.InstNoOp(
                            name=f"wsplit-{_SPLIT_N[0]}", ins=[], outs=[]
                        )
                        nop.engine = inst.engine
                        nop.sync_info = bass_rust.SyncInfo(
                            on_wait=[w], on_update=[]
                        )
                        insts.insert(i, nop)
                        i += 1
                    inst.sync_info = bass_rust.SyncInfo(
                        on_wait=keep, on_update=list(si.on_update)
                    )
                i += 1


class _FastTailTC(tile.TileContext):
    """Tile's kernel tail is drain-with-per-sem-waits + 2 all-engine
    barriers + per-sem clears; the per-sem waits explode into ~70 NOPs per
    engine under the single-wait walrus (~8 us).  All DMAs except the
    output writeback have already been observed by their consumers, so a
    barrier (engines idle, all triggers issued) followed by the range-based
    DMA drain + semaphore clear inside clear_and_free_semaphores is enough."""

    def _drain_and_barrier(self, tick_clock, wait_clock):
        nc = self.nc
        nc.all_engine_barrier()
        popped = nc._tile_sem_poison_stack.pop()
        assert popped is self._sem_poison
        assert self.sems is not None
        nc.clear_and_free_semaphores(list(self.sems.allocated().values()))


def _build_nc():
    f32 = mybir.dt.float32
    dt = mybir.dt.float16
    nc = bass.Bass()

    w_d = [
        nc.dram_tensor(f"w{l}s", [128, NKT * n], dt, kind="ExternalInput")
        for l, n in enumerate(N_L)
    ]
    xt_d = nc.dram_tensor("xt", [128, 512], dt, kind="ExternalInput")
    # [ blT (8x128) | bias0 (8x512) | bias1 (8x512) | bias2 (8x311) ]
    SMALL_COLS = 128 + D1 + D2 + D3
    small_d = nc.dram_tensor("small", [8, SMALL_COLS], dt, kind="ExternalInput")
    # bb[p, e*512 + it*128 + b] = blend[e, b]: DMA-ing this 1 MB broadcast
    # costs ~2.7 us of stream but frees ~4 us of the in-order tensor queue
    # (on-chip K=8 broadcast matmuls measured slower than the DMA)
    bb_d = nc.dram_tensor("bb", [128, E * 512], dt, kind="ExternalInput")
    out_d = nc.dram_tensor("out", [128, D3], f32, kind="ExternalOutput")

    with _FastTailTC(nc) as tc:
        with (
            tc.tile_pool(name="const", bufs=1) as const,
            tc.tile_pool(name="w", bufs=12) as wpool,
            tc.tile_pool(name="acts", bufs=2) as acts,
            tc.tile_pool(name="tmp", bufs=2) as tmp,
            tc.tile_pool(name="zp", bufs=2, space="PSUM") as zp,
            tc.tile_pool(name="tp", bufs=2, space="PSUM") as tp,
        ):
            # everything rides the sync HW-DGE queue: the scalar queue
            # initializes lazily (~3.5 us) and moves 512-byte packets, so
            # it is useless for latency; small inputs go first (~0.6 us)
            small_sb = const.tile([8, SMALL_COLS], dt)
            nc.sync.dma_start(small_sb[:], small_d[:])
            xt_sb = const.tile([128, 512], dt)
            nc.sync.dma_start(xt_sb[:], xt_d[:])

            bl8 = small_sb[:, 0:128]
            bb = const.tile([128, E * 512], dt)
            # first bb half (experts 0-3) ahead of w0g0 so layer-0 expansion
            # overlaps the first weight group's transfer; second half rides
            # between w0g0 and w0g1 (needed only by the third mm group)
            nc.sync.dma_start(bb[:, : E * 256], bb_d[:, : E * 256])
            waug = [None] * 3
            off = 128
            for l, n in enumerate(N_L):
                waug[l] = small_sb[:, off : off + n]
                off += n

            # weight stream: strict consumption order on the sync queue
            wg = [[] for _ in N_L]  # [(tile, start_kt, n_kt)]
            for l, n in enumerate(N_L):
                for g in range(NKT // GSIZE):
                    t = wpool.tile([128, GSIZE * n], dt, tag=f"w{l}g{g}", bufs=1)
                    nc.sync.dma_start(
                        t[:], w_d[l][:, g * GSIZE * n : (g + 1) * GSIZE * n]
                    )
                    wg[l].append((t, g * GSIZE, GSIZE))
                    if l == 0 and g == 0:
                        nc.sync.dma_start(
                            bb[:, E * 256 :], bb_d[:, E * 256 :]
                        )

            ident = const.tile([128, 128], dt)
            make_identity(nc, ident[:])

            def expand(he, src_ap, e, lo, hi):
                # he[:, e*512+lo : e*512+hi] = src_ap * blend[e]
                nc.vector.tensor_tensor(
                    he[:, e * 512 + lo : e * 512 + hi],
                    src_ap,
                    bb[:, e * 512 + lo : e * 512 + hi],
                    mybir.AluOpType.mult,
                )

            he = acts.tile([128, E * 512], dt, tag="he")
            for e in range(E):
                expand(he, xt_sb[:], e, 0, 512)

            for l, n in enumerate(N_L):
                z = zp.tile([128, n], f32, tag="z")
                # bias: z[b, o] = sum_e bl[e,b] * bias[e,o]   (K=8 matmul)
                nc.tensor.matmul(z[:], bl8, waug[l], start=True, stop=False)
                order = _ktile_order(l)
                for j, (e, it) in enumerate(order):
                    g, loc = divmod(j, GSIZE)
                    wt = wg[l][g][0]
                    nc.tensor.matmul(
                        z[:],
                        he[:, e * 512 + it * 128 : e * 512 + (it + 1) * 128],
                        wt[:, loc * n : (loc + 1) * n],
                        start=False,
                        stop=(j == NKT - 1),
                    )

                if l == 2:
                    # output: two halves so copy/trigger/data overlap
                    out_sb = tmp.tile([128, D3], f32, tag="osb")
                    nc.scalar.copy(out_sb[:, 0:160], z[:, 0:160])
                    nc.sync.dma_start(out_d[:, 0:160], out_sb[:, 0:160])
                    nc.vector.tensor_copy(out_sb[:, 160:D3], z[:, 160:D3])
                    nc.sync.dma_start(out_d[:, 160:D3], out_sb[:, 160:D3])
                    break

                # boundary: ELU + transpose + expansion, one 256-col half at
                # a time so layer l+1 matmuls start after the first half.
                he_next = acts.tile([128, E * 512], dt, tag="he")
                for half in range(2):
                    lo, hi = half * 256, (half + 1) * 256
                    m = tmp.tile([128, 256], f32, tag=f"m{half}")
                    nc.vector.tensor_scalar(
                        m[:], z[:, lo:hi], 0.0, None, mybir.AluOpType.min
                    )
                    ex = tmp.tile([128, 256], f32, tag=f"ex{half}")
                    nc.scalar.activation(
                        ex[:], m[:], mybir.ActivationFunctionType.Exp
                    )
                    p = tmp.tile([128, 256], f32, tag=f"p{half}")
                    nc.vector.tensor_scalar(
                        p[:], z[:, lo:hi], 0.0, -1.0,
                        mybir.AluOpType.max, mybir.AluOpType.add,
                    )
                    h = tmp.tile([128, 256], dt, tag=f"h{half}")
                    nc.vector.tensor_tensor(
                        h[:], p[:], ex[:], mybir.AluOpType.add
                    )
                    tps = tp.tile([128, 256], dt, tag=f"tps{half}", bufs=1)
                    for a in range(2):
                        nc.tensor.transpose(
                            tps[:, a * 128 : (a + 1) * 128],
                            h[:, a * 128 : (a + 1) * 128],
                            ident[:],
                        )
                    hT = tmp.tile([128, 256], dt, tag=f"hT{half}")
                    nc.scalar.copy(hT[:], tps[:])
                    for e in range(E):
                        expand(he_next, hT[:], e, lo, hi)
                he = he_next

    _split_multi_waits(nc)
    return nc


# ---- host-side packing -----------------------------------------------------


def _wimg(w, layer, np_dt):
    """(E, dout, din) weights -> (128, 32*dout) SBUF image in the k-tile
    order consumed by the matmul loop of this layer."""
    e_, dout, din = w.shape
    wt = np.zeros((E, 4 * 128, dout), np.float32)
    for e in range(e_):
        wt[e, :din] = w[e].T
    order = _ktile_order(layer)
    img = np.empty((128, NKT * dout), np.float32)
    for j, (e, it) in enumerate(order):
        img[:, j * dout : (j + 1) * dout] = wt[e, it * 128 : (it + 1) * 128]
    return np.ascontiguousarray(img).astype(np_dt)


def kernel(x, weight_blend, w0, b0, w1, b1, w2, b2):
    np_dt = np.float16

    if "nc" not in _NC_CACHE:
        _NC_CACHE["nc"] = _build_nc()
    nc = _NC_CACHE["nc"]

    x = np.asarray(x, np.float32)
    weight_blend = np.asarray(weight_blend, np.float32)
    wimgs = {
        "w0s": _wimg(np.asarray(w0), 0, np_dt),
        "w1s": _wimg(np.asarray(w1), 1, np_dt),
        "w2s": _wimg(np.asarray(w2), 2, np_dt),
    }
    biases = [np.asarray(b, np.float32) for b in (b0, b1, b2)]

    bc = B // N_CORES
    in_maps = []
    for c in range(N_CORES):
        sl = slice(c * bc, (c + 1) * bc)
        xT = np.zeros((4 * 128, bc), np.float32)
        xT[:DIN] = x[sl].T
        xt_img = xT.reshape(4, 128, bc).transpose(1, 0, 2).reshape(128, 4 * bc)
        bl = weight_blend[:, sl]  # (8, 128)
        small_img = np.concatenate([bl] + biases, axis=1)
        bb_img = np.broadcast_to(
            bl[:, None, None, :], (E, 4, 128, bc)
        ).transpose(2, 0, 1, 3).reshape(128, E * 4 * bc)
        in_maps.append(
            {
                **wimgs,
                "xt": np.ascontiguousarray(xt_img).astype(np_dt),
                "small": np.ascontiguousarray(small_img).astype(np_dt),
                "bb": np.ascontiguousarray(bb_img).astype(np_dt),
            }
        )

    res = run_bass_kernel_spmd(
        nc,
        in_maps,
        core_ids=list(range(N_CORES)),
        trace=PROFILE["trace"],
        tmpdir=PROFILE["tmpdir"],
    )
    LAST_RESULT[0] = res
    return np.concatenate(
        [res.results[c]["out"] for c in range(N_CORES)], axis=0
    )
